# revision 11
# baseline (speedup 1.0000x reference)
"""DeepSet segment-reduce kernel for 8 Trainium2 NeuronCores (Bass/Tile).

Math (reference):
    h  = relu(x1 @ W1 + b1) @ W2 + b2          # [E, 128]
    S  = segment_sum(h, seg)                   # [B, 128]
    mean = S / max(counts, 1)
    out  = mean @ W3 + b3

Because segment-sum is linear, only r = relu(x1 @ W1 + b1) needs per-edge
work:  segsum(h) = segsum(r) @ W2 + counts x b2.

Per-core streaming is HBM-bound at f32 (512 MB of x1), so x1 is cast to
bf16 on the host (rel err ~3e-3 « 2e-2 gate), halving DMA bytes.  The
per-element post-matmul work is then the wall; it is split so no single
engine exceeds the DMA floor:

  host: repack edges so every segment starts on a 32-edge block boundary
        (pad columns use x_pad with W1.T x_pad + b1 < 0, so relu kills
        them exactly), shard contiguous segment runs across 8 cores
        balanced by block count; precompute the one-hot block->local-
        segment matrices A (bf16) so no on-device index math is needed.
  core: stream xT [128, E_cap] bf16 tiles; hT = W1.T @ xT (PE, bf16 W1
        stationary) into [128, 1536] PSUM tiles; bias+relu PSUM->SBUF
        (bf16 out) fused on ACT (~78% of tiles, activation w/ bias) or
        DVE (~22%, tensor_scalar add+max) -- ratio balances the engines;
        per-32-block sums via a pairwise bf16 tensor_tensor tree on DVE
        (2x perf mode; tensor_reduce is capped at 1x), final level f32.
        The tree for super-tile s is emitted after super-tile s+1's DVE
        relu slots so the PSUM pipeline never queues behind a tree burst.
        ST[f, b_local] = block_sumsT @ A via PE transposes + matmuls
        (identical instruction stream on all cores -> one SPMD program);
        every segment is wholly owned by one core (contiguous runs), so
        the per-core partial ST is already final -> no collective; each
        core runs the small W2/W3 stage on its W=256 local window only:
        sums_hT = W2.T @ S + b2 x counts (rank-1 via k=1 matmul);
        meanT   = sums_hT * inv_counts (broadcast tensor input);
        out     = meanT.T @ W3 + 1 x b3 (rank-1), DMA to [W, 128] local
        output; host stitches the 8 disjoint output row ranges.

Self-contained: no reads of /root/problem/*; shapes derived from inputs.
"""

import math

import numpy as np

N_CORES = 8
BLOCK = 32           # segment alignment quantum (edges per block)
PSUM_TILE = 1024     # columns per PSUM tile / relu op (2 banks)
DMA_TILE = 6144      # xT columns per DMA (1.5 MiB bf16)
SUPER = 12288        # columns per block-sum tree pass (12 PSUM tiles)
WINDOW = 256         # per-core local segment window (>= max run + pad)
MM_N = 512           # matmul free-dim chunk (1 PSUM bank of f32)
PAD_MARGIN = 8.0


def _bf16(a):
    import ml_dtypes

    return np.asarray(a).astype(ml_dtypes.bfloat16)


def _plan_shards(edge_slices, E, B):
    es = np.asarray(edge_slices, dtype=np.int64)
    counts = (es[1:] - es[:-1]).astype(np.int64)        # [B]
    seg_blocks = (counts + BLOCK - 1) // BLOCK          # [B]
    total_blocks = int(seg_blocks.sum())

    # contiguous runs of segments per core, balanced by block count
    cum = np.cumsum(seg_blocks)
    bounds = [0]
    for c in range(1, N_CORES):
        bounds.append(int(np.searchsorted(cum, c * total_blocks / N_CORES)))
    bounds.append(B)

    core_blocks = []
    for c in range(N_CORES):
        core_blocks.append(int(seg_blocks[bounds[c]:bounds[c + 1]].sum()))
        assert bounds[c + 1] - bounds[c] <= WINDOW, (
            "segment run exceeds local window", bounds)
    j_max = max(core_blocks)
    e_cap = int(math.ceil(j_max * BLOCK / PSUM_TILE) * PSUM_TILE)
    return es, counts, seg_blocks, bounds, e_cap


def _solve_xpad(W1bf, b1):
    # x_pad (bf16) with W1.T x_pad + b1 <= -1 elementwise => relu output 0
    W1d = W1bf.astype(np.float64)
    margin = PAD_MARGIN
    for _ in range(6):
        rhs = -(b1.astype(np.float64) + margin)
        x_pad = _bf16(np.linalg.solve(W1d.T, rhs))
        chk = W1d.T @ x_pad.astype(np.float64) + b1.astype(np.float64)
        if chk.max() < -1.0:
            return x_pad
        margin *= 2.0
    raise AssertionError("x_pad margin too small")


def _build_core_inputs(x1bf, es, counts, seg_blocks, bounds, e_cap, x_pad, B):
    J = e_cap // BLOCK
    n_chunks = (J + 127) // 128
    xTs, amaps = [], []
    for c in range(N_CORES):
        xT = np.empty((128, e_cap), dtype=x1bf.dtype)
        segid = np.full(n_chunks * 128, -1, dtype=np.int64)
        pos = 0
        for b in range(bounds[c], bounds[c + 1]):
            cnt = int(counts[b])
            if cnt == 0:
                continue
            xT[:, pos:pos + cnt] = x1bf[es[b]:es[b + 1], :].T
            nb = int(seg_blocks[b])
            pad = nb * BLOCK - cnt
            if pad:
                xT[:, pos + cnt:pos + nb * BLOCK] = x_pad[:, None]
            segid[pos // BLOCK: pos // BLOCK + nb] = b - bounds[c]
            pos += nb * BLOCK
        if pos < e_cap:
            xT[:, pos:] = x_pad[:, None]
        # one-hot A: block (chunk k, row p) -> local segment column
        amap = np.zeros((128, n_chunks * WINDOW), np.float32)
        for k in range(n_chunks):
            sid = segid[k * 128:(k + 1) * 128]
            p = np.nonzero(sid >= 0)[0]
            amap[p, k * WINDOW + sid[p]] = 1.0
        xTs.append(xT)
        amaps.append(_bf16(amap))
    return xTs, amaps, J


def _dve_slots(s):
    # DVE-relu sub-tile positions within super-tile s (rest go to ACT);
    # spread out; avg 2.5/12 balances ACT(relu) vs DVE(relu+tree).
    return (2, 5, 11) if s % 2 else (5, 11)


def _build_bass(e_cap, J, B, repeat=1):
    n_chunks = (J + 127) // 128
    n_batches = (n_chunks + 3) // 4
    W = WINDOW
    import concourse.bacc as bacc
    import concourse.mybir as mybir
    import concourse.tile as tile

    f32 = mybir.dt.float32
    f32r = mybir.dt.float32r
    bf16 = mybir.dt.bfloat16
    Relu = mybir.ActivationFunctionType.Relu
    Add = mybir.AluOpType.add
    Max = mybir.AluOpType.max

    nc = bacc.Bacc(trn_type="TRN2", num_devices=N_CORES)

    xT_d = nc.dram_tensor("xT", [128, e_cap], bf16, kind="ExternalInput")
    A_d = nc.dram_tensor("amap", [128, n_chunks * W], bf16,
                         kind="ExternalInput")
    W1_d = nc.dram_tensor("W1b", [128, 128], bf16, kind="ExternalInput")
    b1_d = nc.dram_tensor("b1c", [128, 1], f32, kind="ExternalInput")
    W2_d = nc.dram_tensor("W2", [128, 128], f32r, kind="ExternalInput")
    b2_d = nc.dram_tensor("b2r", [1, 128], f32r, kind="ExternalInput")
    W3_d = nc.dram_tensor("W3", [128, 128], f32, kind="ExternalInput")
    b3_d = nc.dram_tensor("b3r", [1, 128], f32, kind="ExternalInput")
    cnt_d = nc.dram_tensor("counts_row", [1, W], f32r, kind="ExternalInput")
    inv_d = nc.dram_tensor("inv_bcast", [128, W], f32, kind="ExternalInput")
    ones_d = nc.dram_tensor("ones_row", [1, 128], f32, kind="ExternalInput")
    ident_d = nc.dram_tensor("ident", [128, 128], f32, kind="ExternalInput")
    out_d = nc.dram_tensor("out", [W, 128], f32, kind="ExternalOutput")

    n_super = (e_cap + SUPER - 1) // SUPER

    with tile.TileContext(nc) as tc, tc.tile_pool(name="persist", bufs=1):
        prev_last = [None]

        def emit_body(rep, bp):
          # one full pass of the kernel; rep > 0 only exists for the
          # repeat-timing harness (same work re-emitted, serialized on rep-1)
          sx = f"_{rep}" if rep else ""

          def dep_on_prev(inst):
              if prev_last[0] is not None:
                  from concourse.tile_rust import add_dep_helper
                  add_dep_helper(inst.ins, prev_last[0].ins, sync=True,
                                 reason="repeat-timing serialization")

          w1_sb = bp.tile([128, 128], bf16, name=f"w1_sb{sx}")
          b1_sb = bp.tile([128, 1], f32, name=f"b1_sb{sx}")
          ident_sb = bp.tile([128, 128], f32, name=f"ident_sb{sx}")
          amap_sb = bp.tile([128, n_chunks * W], bf16, name=f"amap_sb{sx}")
          warm_sb = bp.tile([128, 1], f32, name=f"warm_sb{sx}")

          with (
              tc.tile_pool(name=f"xp{sx}", bufs=4) as xp,
              tc.tile_pool(name=f"rp{sx}", bufs=3) as rp,
              tc.tile_pool(name=f"sc{sx}", bufs=1) as scp,
              tc.tile_pool(name=f"hp{sx}", bufs=3, space="PSUM") as hp,
              tc.tile_pool(name=f"tp{sx}", bufs=1, space="PSUM") as tp,
              tc.tile_pool(name=f"stp{sx}", bufs=1, space="PSUM") as stp,
          ):
            st_ps = stp.tile([128, W], f32, name=f"st_ps{sx}")
            scrA = scp.tile([128, SUPER // 2], bf16, name=f"scrA{sx}")
            scrB = scp.tile([128, SUPER // 4], bf16, name=f"scrB{sx}")
            bs_sb = scp.tile([128, n_chunks * 128], f32, name=f"bs_sb{sx}")
            bsT_sb = scp.tile([128, n_batches * 512], bf16,
                              name=f"bsT_sb{sx}")

            def emit_batch(b):
                # 4 PE transposes of bs chunks -> one PSUM tile -> one copy
                # to bsT (bf16), then the block->segment matmuls.
                k0, k1 = b * 4, min(b * 4 + 4, n_chunks)
                tpb = tp.tile([128, 512], f32, name=f"tpb{sx}")
                for k in range(k0, k1):
                    jw = min(128, J - k * 128)
                    s = (k - k0) * 128
                    nc.tensor.transpose(
                        tpb[:jw, s:s + 128],
                        bs_sb[:, k * 128:k * 128 + jw], ident_sb[:])
                if b % 2:
                    nc.scalar.copy(bsT_sb[:, b * 512:(b + 1) * 512], tpb[:])
                else:
                    nc.vector.tensor_copy(
                        bsT_sb[:, b * 512:(b + 1) * 512], tpb[:])
                for k in range(k0, k1):
                    jw = min(128, J - k * 128)
                    nc.tensor.matmul(
                        st_ps[:, 0:W],
                        lhsT=bsT_sb[:jw, k * 128:(k + 1) * 128],
                        rhs=amap_sb[:jw, k * W:(k + 1) * W],
                        start=(k == 0), stop=(k == n_chunks - 1),
                    )

            def emit_tree(rt, c0, c1, j0):
                # per-32-block sums over rt[:, c0:c1]: pairwise adds, bf16
                # intermediates at 2x DVE rate, final level f32 into bs_sb
                j = (c1 - c0) // BLOCK

                def lvl(src, dst, w, n):
                    si = src.rearrange("p (j w) -> p j w", w=w)[:, 0:n, :]
                    do = dst.rearrange(
                        "p (j w) -> p j w", w=w // 2)[:, 0:n, :]
                    nc.vector.tensor_tensor(
                        do, si[:, :, 0:w // 2], si[:, :, w // 2:w], op=Add)

                lvl(rt[:, c0:c1], scrA[:, 0:j * 16], 32, j)
                lvl(scrA[:, 0:j * 16], scrB[:, 0:j * 8], 16, j)
                lvl(scrB[:, 0:j * 8], scrA[:, 0:j * 4], 8, j)
                lvl(scrA[:, 0:j * 4], scrB[:, 0:j * 2], 4, j)
                s2 = scrB[:, 0:j * 2].rearrange("p (j w) -> p j w", w=2)
                bsw = bs_sb[:, j0:j0 + j].rearrange("p (j w) -> p j w", w=1)
                nc.vector.tensor_tensor(
                    bsw, s2[:, :, 0:1], s2[:, :, 1:2], op=Add)

            next_batch = [0]

            def emit_ready_batches(blocks_done, force=False):
                while next_batch[0] < n_batches:
                    b = next_batch[0]
                    hi = min((b * 4 + 4) * 128, J)
                    if not force and blocks_done < hi:
                        break
                    emit_batch(b)
                    next_batch[0] += 1

            pend_tree = []   # (rt, c0, c1, j0) lagged one super-tile

            for s in range(n_super):
                s0 = s * SUPER
                scols = min(SUPER, e_cap - s0)
                rt = rp.tile([128, SUPER], bf16, name=f"rt{sx}")
                dve = _dve_slots(s)
                for dloc in range(0, scols, DMA_TILE):
                    xt = xp.tile([128, DMA_TILE], bf16, name=f"xt{sx}")
                    dcols = min(DMA_TILE, scols - dloc)
                    if s == 0 and dloc == 0:
                        # load weights first + trigger ACT table load early
                        wd = nc.sync.dma_start(w1_sb[:], W1_d[:])
                        dep_on_prev(wd)
                        wd = nc.sync.dma_start(b1_sb[:], b1_d[:])
                        dep_on_prev(wd)
                        nc.scalar.activation(warm_sb[:], b1_sb[:], Relu)
                    # first super: small starter DMAs to fill the pipe fast
                    step = PSUM_TILE if (s == 0 and dloc == 0) else dcols
                    for d0 in range(0, dcols, step):
                        di = nc.sync.dma_start(
                            xt[:, d0:d0 + step],
                            xT_d[:, s0 + dloc + d0:s0 + dloc + d0 + step])
                        dep_on_prev(di)
                    if s == 0 and dloc == 0:
                        nc.sync.dma_start(ident_sb[:], ident_d[:])
                    for hloc in range(0, dcols, PSUM_TILE):
                        ps = hp.tile([128, PSUM_TILE], f32, name=f"ps{sx}")
                        for q in range(PSUM_TILE // MM_N):
                            c0 = hloc + q * MM_N
                            nc.tensor.matmul(
                                ps[:, q * MM_N:(q + 1) * MM_N],
                                lhsT=w1_sb[:],
                                rhs=xt[:, c0:c0 + MM_N],
                                start=True, stop=True,
                            )
                        dst = rt[:, dloc + hloc:dloc + hloc + PSUM_TILE]
                        h = (dloc + hloc) // PSUM_TILE
                        if h in dve:
                            nc.vector.tensor_scalar(
                                dst, ps[:], b1_sb[:, 0:1], 0.0,
                                op0=Add, op1=Max)
                        else:
                            nc.scalar.activation(
                                dst, ps[:], Relu, bias=b1_sb[:, 0:1])
                    if s == 0 and dloc + DMA_TILE >= scols:
                        # SWDGE queue after super 0 is underway: off the
                        # HWDGE FIFO and past the pipeline-fill window;
                        # first consumer (batch 0) runs ~2 supers later
                        nc.gpsimd.dma_start(amap_sb[:], A_d[:])
                    if s == n_super - 1:
                        # tail: drain the lagged tree early, then eager
                        # per-DMA-tile trees so little work trails the
                        # final relu
                        while pend_tree:
                            emit_tree(*pend_tree.pop(0))
                            emit_ready_batches(s0 // BLOCK)
                        emit_tree(rt, dloc, dloc + dcols,
                                  (s0 + dloc) // BLOCK)
                if s < n_super - 1:
                    # emit the LAGGED tree (for super s-1) after this
                    # super's relu work is queued, so DVE relus never
                    # wait behind a tree burst
                    while pend_tree:
                        emit_tree(*pend_tree.pop(0))
                        emit_ready_batches(s0 // BLOCK)
                    pend_tree.append((rt, 0, scols, s0 // BLOCK))
            emit_ready_batches(J, force=True)

            # every segment is wholly owned by one core (contiguous runs),
            # so the per-core partial ST is already the FINAL sum for this
            # core's segment window: no collective needed.
            sfull_sb = bp.tile([128, W], f32r, name=f"sfull_sb{sx}")
            nc.scalar.copy(sfull_sb[:], st_ps[:])

          # final: sums_hT = W2.T @ S + b2 x counts; meanT; out
          w2_sb = bp.tile([128, 128], f32r, name=f"w2_sb{sx}")
          b2_sb = bp.tile([1, 128], f32r, name=f"b2_sb{sx}")
          w3_sb = bp.tile([128, 128], f32, name=f"w3_sb{sx}")
          b3_sb = bp.tile([1, 128], f32, name=f"b3_sb{sx}")
          cnt_sb = bp.tile([1, W], f32r, name=f"cnt_sb{sx}")
          inv_sb = bp.tile([128, W], f32, name=f"inv_sb{sx}")
          ones_sb = bp.tile([1, 128], f32, name=f"ones_sb{sx}")
          mean_sb = bp.tile([128, W], f32, name=f"mean_sb{sx}")
          nc.sync.dma_start(w2_sb[:], W2_d[:])
          nc.sync.dma_start(b2_sb[:], b2_d[:])
          nc.sync.dma_start(w3_sb[:], W3_d[:])
          nc.sync.dma_start(b3_sb[:], b3_d[:])
          nc.sync.dma_start(cnt_sb[:], cnt_d[:])
          nc.sync.dma_start(inv_sb[:], inv_d[:])
          nc.sync.dma_start(ones_sb[:], ones_d[:])

          with (
              tc.tile_pool(name=f"sp{sx}", bufs=1, space="PSUM") as sp,
              tc.tile_pool(name=f"op{sx}", bufs=1, space="PSUM") as op,
              tc.tile_pool(name=f"op_sb{sx}", bufs=1) as op_sb,
          ):
            sums_ps = sp.tile([128, W], f32, name=f"sums_ps{sx}")
            nc.tensor.matmul(sums_ps[:, 0:W], lhsT=w2_sb[:],
                             rhs=sfull_sb[:, 0:W], start=True, stop=False)
            nc.tensor.matmul(sums_ps[:, 0:W], lhsT=b2_sb[0:1, :],
                             rhs=cnt_sb[0:1, 0:W], start=False, stop=True)
            nc.vector.tensor_mul(mean_sb[:], sums_ps[:], inv_sb[:])

            n_oc = W // 128
            ops = op.tile([128, n_oc * 128], f32, name=f"ops{sx}")
            for c in range(n_oc):
                c0 = c * 128
                nc.tensor.matmul(ops[:, c0:c0 + 128],
                                 lhsT=mean_sb[:, c0:c0 + 128],
                                 rhs=w3_sb[:], start=True, stop=False)
                nc.tensor.matmul(ops[:, c0:c0 + 128],
                                 lhsT=ones_sb[0:1, :],
                                 rhs=b3_sb[0:1, :], start=False, stop=True)
            osb = op_sb.tile([128, n_oc * 128], f32, name=f"osb{sx}")
            nc.scalar.copy(osb[:], ops[:])
            for c in range(n_oc):
                c0 = c * 128
                last = nc.sync.dma_start(out_d[c0:c0 + 128, :],
                                         osb[:, c0:c0 + 128])
            prev_last[0] = last

        for rep in range(repeat):
            with tc.tile_pool(name=f"body_{rep}", bufs=1) as bp:
                emit_body(rep, bp)

    nc.compile()
    return nc


def _prepare(x1, edge_slices, W1, b1, W2, b2, W3, b3):
    """Host planning + per-core input construction + Bass program build."""
    x1bf = _bf16(np.ascontiguousarray(np.asarray(x1, dtype=np.float32)))
    W1 = np.asarray(W1, dtype=np.float32)
    b1 = np.asarray(b1, dtype=np.float32)
    E = x1bf.shape[0]
    B = int(np.asarray(edge_slices).shape[0]) - 1

    es, counts, seg_blocks, bounds, e_cap = _plan_shards(edge_slices, E, B)
    W1bf = _bf16(W1)
    x_pad = _solve_xpad(W1bf, b1)
    xTs, amaps, J = _build_core_inputs(x1bf, es, counts, seg_blocks, bounds,
                                       e_cap, x_pad, B)

    counts_f = counts.astype(np.float32)
    inv = (1.0 / np.maximum(counts_f, 1.0)).astype(np.float32)
    Wn = WINDOW
    shared = {
        "W1b": W1bf,
        "b1c": np.ascontiguousarray(b1.reshape(128, 1)),
        "W2": np.asarray(W2, dtype=np.float32),
        "b2r": np.ascontiguousarray(np.asarray(b2, np.float32).reshape(1, 128)),
        "W3": np.asarray(W3, dtype=np.float32),
        "b3r": np.ascontiguousarray(np.asarray(b3, np.float32).reshape(1, 128)),
        "ones_row": np.ones((1, 128), np.float32),
        "ident": np.eye(128, dtype=np.float32),
    }

    nc = _build_bass(e_cap, J, B)
    in_maps = []
    for c in range(N_CORES):
        n_c = bounds[c + 1] - bounds[c]
        cnt_w = np.zeros((1, Wn), np.float32)
        cnt_w[0, :n_c] = counts_f[bounds[c]:bounds[c + 1]]
        inv_w = np.zeros((128, Wn), np.float32)
        inv_w[:, :n_c] = inv[None, bounds[c]:bounds[c + 1]]
        in_maps.append({
            "xT": xTs[c], "amap": amaps[c],
            "counts_row": cnt_w, "inv_bcast": np.ascontiguousarray(inv_w),
            **shared,
        })
    return nc, in_maps, bounds


def _assemble(outs, bounds, B):
    out = np.empty((B, 128), dtype=np.float32)
    for c in range(N_CORES):
        n_c = bounds[c + 1] - bounds[c]
        out[bounds[c]:bounds[c + 1], :] = outs[c][:n_c, :]
    return out


def kernel(x1, edge_slices, W1, b1, W2, b2, W3, b3):
    from concourse import bass_utils

    nc, in_maps, bounds = _prepare(x1, edge_slices, W1, b1, W2, b2, W3, b3)
    br = bass_utils.run_bass_kernel_spmd(
        nc, in_maps, core_ids=list(range(N_CORES))
    )
    B = int(np.asarray(edge_slices).shape[0]) - 1
    return _assemble([r["out"] for r in br.results], bounds, B)


# revision 12
# speedup vs baseline: 1.2152x; 1.2152x over previous
"""DeepSet segment-reduce kernel for 8 Trainium2 NeuronCores (Bass/Tile).

Math (reference):
    h  = relu(x1 @ W1 + b1) @ W2 + b2          # [E, 128]
    S  = segment_sum(h, seg)                   # [B, 128]
    mean = S / max(counts, 1)
    out  = mean @ W3 + b3

Because segment-sum is linear, only r = relu(x1 @ W1 + b1) needs per-edge
work:  segsum(h) = segsum(r) @ W2 + counts x b2.

Per-core streaming is HBM-bound at f32 (512 MB of x1), so x1 is cast to
bf16 on the host (rel err ~3e-3 « 2e-2 gate), halving DMA bytes.  The
per-element post-matmul work is then the wall; it is split so no single
engine exceeds the DMA floor:

  host: repack edges so every segment starts on a 32-edge block boundary
        (pad columns use x_pad with W1.T x_pad + b1 < 0, so relu kills
        them exactly), shard contiguous segment runs across 8 cores
        balanced by block count; precompute the one-hot block->local-
        segment matrices A (bf16) so no on-device index math is needed.
  core: stream xT [128, E_cap] bf16 tiles; hT = W1.T @ xT (PE, bf16 W1
        stationary) into [128, 1536] PSUM tiles; bias+relu PSUM->SBUF
        (bf16 out) fused on ACT (~78% of tiles, activation w/ bias) or
        DVE (~22%, tensor_scalar add+max) -- ratio balances the engines;
        per-32-block sums via a pairwise bf16 tensor_tensor tree on DVE
        (2x perf mode; tensor_reduce is capped at 1x), final level f32.
        The tree for super-tile s is emitted after super-tile s+1's DVE
        relu slots so the PSUM pipeline never queues behind a tree burst.
        ST[f, b_local] = block_sumsT @ A via PE transposes + matmuls
        (identical instruction stream on all cores -> one SPMD program);
        every segment is wholly owned by one core (contiguous runs), so
        the per-core partial ST is already final -> no collective; each
        core runs the small W2/W3 stage on its W=256 local window only:
        sums_hT = W2.T @ S + b2 x counts (rank-1 via k=1 matmul);
        meanT   = sums_hT * inv_counts (broadcast tensor input);
        out     = meanT.T @ W3 + 1 x b3 (rank-1), DMA to [W, 128] local
        output; host stitches the 8 disjoint output row ranges.

Self-contained: no reads of /root/problem/*; shapes derived from inputs.
"""

import math

import numpy as np

N_CORES = 8
BLOCK = 32           # segment alignment quantum (edges per block)
PSUM_TILE = 1024     # columns per PSUM tile / relu op (2 banks)
DMA_TILE = 6144      # xT columns per DMA (1.5 MiB bf16)
SUPER = 12288        # columns per block-sum tree pass (12 PSUM tiles)
WINDOW = 256         # per-core local segment window (>= max run + pad)
MM_N = 512           # matmul free-dim chunk (1 PSUM bank of f32)
PAD_MARGIN = 8.0


def _bf16(a):
    import ml_dtypes

    return np.asarray(a).astype(ml_dtypes.bfloat16)


def _plan_shards(edge_slices, E, B):
    es = np.asarray(edge_slices, dtype=np.int64)
    counts = (es[1:] - es[:-1]).astype(np.int64)        # [B]
    seg_blocks = (counts + BLOCK - 1) // BLOCK          # [B]
    total_blocks = int(seg_blocks.sum())

    # contiguous runs of segments per core, balanced by block count
    cum = np.cumsum(seg_blocks)
    bounds = [0]
    for c in range(1, N_CORES):
        bounds.append(int(np.searchsorted(cum, c * total_blocks / N_CORES)))
    bounds.append(B)

    core_blocks = []
    for c in range(N_CORES):
        core_blocks.append(int(seg_blocks[bounds[c]:bounds[c + 1]].sum()))
        assert bounds[c + 1] - bounds[c] <= WINDOW, (
            "segment run exceeds local window", bounds)
    j_max = max(core_blocks)
    e_cap = int(math.ceil(j_max * BLOCK / PSUM_TILE) * PSUM_TILE)
    return es, counts, seg_blocks, bounds, e_cap


def _solve_xpad(W1bf, b1):
    # x_pad (bf16) with W1.T x_pad + b1 <= -1 elementwise => relu output 0
    W1d = W1bf.astype(np.float64)
    margin = PAD_MARGIN
    for _ in range(6):
        rhs = -(b1.astype(np.float64) + margin)
        x_pad = _bf16(np.linalg.solve(W1d.T, rhs))
        chk = W1d.T @ x_pad.astype(np.float64) + b1.astype(np.float64)
        if chk.max() < -1.0:
            return x_pad
        margin *= 2.0
    raise AssertionError("x_pad margin too small")


def _build_core_inputs(x1bf, es, counts, seg_blocks, bounds, e_cap, x_pad, B):
    J = e_cap // BLOCK
    n_chunks = (J + 127) // 128
    xTs, amaps = [], []
    for c in range(N_CORES):
        xT = np.empty((128, e_cap), dtype=x1bf.dtype)
        segid = np.full(n_chunks * 128, -1, dtype=np.int64)
        pos = 0
        for b in range(bounds[c], bounds[c + 1]):
            cnt = int(counts[b])
            if cnt == 0:
                continue
            xT[:, pos:pos + cnt] = x1bf[es[b]:es[b + 1], :].T
            nb = int(seg_blocks[b])
            pad = nb * BLOCK - cnt
            if pad:
                xT[:, pos + cnt:pos + nb * BLOCK] = x_pad[:, None]
            segid[pos // BLOCK: pos // BLOCK + nb] = b - bounds[c]
            pos += nb * BLOCK
        if pos < e_cap:
            xT[:, pos:] = x_pad[:, None]
        # one-hot A: block (chunk k, row p) -> local segment column
        amap = np.zeros((128, n_chunks * WINDOW), np.float32)
        for k in range(n_chunks):
            sid = segid[k * 128:(k + 1) * 128]
            p = np.nonzero(sid >= 0)[0]
            amap[p, k * WINDOW + sid[p]] = 1.0
        xTs.append(xT)
        amaps.append(_bf16(amap))
    return xTs, amaps, J


def _dve_slots(s):
    # DVE-relu sub-tile positions within super-tile s (rest go to ACT);
    # spread out; avg 2.5/12 balances ACT(relu) vs DVE(relu+tree).
    return (2, 5, 11) if s % 2 else (5, 11)


def _build_bass(e_cap, J, B, repeat=1):
    n_chunks = (J + 127) // 128
    n_batches = (n_chunks + 3) // 4
    W = WINDOW
    import concourse.bacc as bacc
    import concourse.mybir as mybir
    import concourse.tile as tile

    f32 = mybir.dt.float32
    f32r = mybir.dt.float32r
    bf16 = mybir.dt.bfloat16
    Relu = mybir.ActivationFunctionType.Relu
    Add = mybir.AluOpType.add
    Max = mybir.AluOpType.max

    nc = bacc.Bacc(trn_type="TRN2", num_devices=N_CORES)

    xT_d = nc.dram_tensor("xT", [128, e_cap], bf16, kind="ExternalInput")
    A_d = nc.dram_tensor("amap", [128, n_chunks * W], bf16,
                         kind="ExternalInput")
    W1_d = nc.dram_tensor("W1b", [128, 128], bf16, kind="ExternalInput")
    b1_d = nc.dram_tensor("b1c", [128, 1], f32, kind="ExternalInput")
    W2_d = nc.dram_tensor("W2", [128, 128], f32r, kind="ExternalInput")
    b2_d = nc.dram_tensor("b2r", [1, 128], f32r, kind="ExternalInput")
    W3_d = nc.dram_tensor("W3", [128, 128], f32, kind="ExternalInput")
    b3_d = nc.dram_tensor("b3r", [1, 128], f32, kind="ExternalInput")
    cnt_d = nc.dram_tensor("counts_row", [1, W], f32r, kind="ExternalInput")
    inv_d = nc.dram_tensor("inv_bcast", [128, W], f32, kind="ExternalInput")
    ones_d = nc.dram_tensor("ones_row", [1, 128], f32, kind="ExternalInput")
    ident_d = nc.dram_tensor("ident", [128, 128], f32, kind="ExternalInput")
    out_d = nc.dram_tensor("out", [W, 128], f32, kind="ExternalOutput")

    n_super = (e_cap + SUPER - 1) // SUPER

    with tile.TileContext(nc) as tc, tc.tile_pool(name="persist", bufs=1):
        prev_last = [None]

        def emit_body(rep, bp):
          # one full pass of the kernel; rep > 0 only exists for the
          # repeat-timing harness (same work re-emitted, serialized on rep-1)
          sx = f"_{rep}" if rep else ""

          def dep_on_prev(inst):
              if prev_last[0] is not None:
                  from concourse.tile_rust import add_dep_helper
                  add_dep_helper(inst.ins, prev_last[0].ins, sync=True,
                                 reason="repeat-timing serialization")

          w1_sb = bp.tile([128, 128], bf16, name=f"w1_sb{sx}")
          b1_sb = bp.tile([128, 1], f32, name=f"b1_sb{sx}")
          ident_sb = bp.tile([128, 128], f32, name=f"ident_sb{sx}")
          amap_sb = bp.tile([128, n_chunks * W], bf16, name=f"amap_sb{sx}")
          warm_sb = bp.tile([128, 1], f32, name=f"warm_sb{sx}")

          with (
              tc.tile_pool(name=f"xp{sx}", bufs=4) as xp,
              tc.tile_pool(name=f"rp{sx}", bufs=3) as rp,
              tc.tile_pool(name=f"sc{sx}", bufs=1) as scp,
              tc.tile_pool(name=f"hp{sx}", bufs=3, space="PSUM") as hp,
              tc.tile_pool(name=f"tp{sx}", bufs=1, space="PSUM") as tp,
              tc.tile_pool(name=f"stp{sx}", bufs=1, space="PSUM") as stp,
          ):
            st_ps = stp.tile([128, W], f32, name=f"st_ps{sx}")
            scrA = scp.tile([128, SUPER // 2], bf16, name=f"scrA{sx}")
            scrB = scp.tile([128, SUPER // 4], bf16, name=f"scrB{sx}")
            bs_sb = scp.tile([128, n_chunks * 128], f32, name=f"bs_sb{sx}")
            bsT_sb = scp.tile([128, n_batches * 512], bf16,
                              name=f"bsT_sb{sx}")

            def emit_batch(b):
                # 4 PE transposes of bs chunks -> one PSUM tile -> one copy
                # to bsT (bf16), then the block->segment matmuls.
                k0, k1 = b * 4, min(b * 4 + 4, n_chunks)
                tpb = tp.tile([128, 512], f32, name=f"tpb{sx}")
                for k in range(k0, k1):
                    jw = min(128, J - k * 128)
                    s = (k - k0) * 128
                    nc.tensor.transpose(
                        tpb[:jw, s:s + 128],
                        bs_sb[:, k * 128:k * 128 + jw], ident_sb[:])
                if b % 2:
                    nc.scalar.copy(bsT_sb[:, b * 512:(b + 1) * 512], tpb[:])
                else:
                    nc.vector.tensor_copy(
                        bsT_sb[:, b * 512:(b + 1) * 512], tpb[:])
                for k in range(k0, k1):
                    jw = min(128, J - k * 128)
                    nc.tensor.matmul(
                        st_ps[:, 0:W],
                        lhsT=bsT_sb[:jw, k * 128:(k + 1) * 128],
                        rhs=amap_sb[:jw, k * W:(k + 1) * W],
                        start=(k == 0), stop=(k == n_chunks - 1),
                    )

            def emit_tree(rt, c0, c1, j0):
                # per-32-block sums over rt[:, c0:c1]: pairwise adds, bf16
                # intermediates at 2x DVE rate, final level f32 into bs_sb
                j = (c1 - c0) // BLOCK

                def lvl(src, dst, w, n):
                    si = src.rearrange("p (j w) -> p j w", w=w)[:, 0:n, :]
                    do = dst.rearrange(
                        "p (j w) -> p j w", w=w // 2)[:, 0:n, :]
                    nc.vector.tensor_tensor(
                        do, si[:, :, 0:w // 2], si[:, :, w // 2:w], op=Add)

                lvl(rt[:, c0:c1], scrA[:, 0:j * 16], 32, j)
                lvl(scrA[:, 0:j * 16], scrB[:, 0:j * 8], 16, j)
                lvl(scrB[:, 0:j * 8], scrA[:, 0:j * 4], 8, j)
                lvl(scrA[:, 0:j * 4], scrB[:, 0:j * 2], 4, j)
                s2 = scrB[:, 0:j * 2].rearrange("p (j w) -> p j w", w=2)
                bsw = bs_sb[:, j0:j0 + j].rearrange("p (j w) -> p j w", w=1)
                nc.vector.tensor_tensor(
                    bsw, s2[:, :, 0:1], s2[:, :, 1:2], op=Add)

            next_batch = [0]

            def emit_ready_batches(blocks_done, force=False):
                while next_batch[0] < n_batches:
                    b = next_batch[0]
                    hi = min((b * 4 + 4) * 128, J)
                    if not force and blocks_done < hi:
                        break
                    emit_batch(b)
                    next_batch[0] += 1

            pend_tree = []   # (rt, c0, c1, j0) lagged one super-tile

            for s in range(n_super):
                s0 = s * SUPER
                scols = min(SUPER, e_cap - s0)
                rt = rp.tile([128, SUPER], bf16, name=f"rt{sx}")
                dve = _dve_slots(s)
                for dloc in range(0, scols, DMA_TILE):
                    xt = xp.tile([128, DMA_TILE], bf16, name=f"xt{sx}")
                    dcols = min(DMA_TILE, scols - dloc)
                    if s == 0 and dloc == 0:
                        # load weights first + trigger ACT table load early
                        wd = nc.sync.dma_start(w1_sb[:], W1_d[:])
                        dep_on_prev(wd)
                        wd = nc.sync.dma_start(b1_sb[:], b1_d[:])
                        dep_on_prev(wd)
                        nc.scalar.activation(warm_sb[:], b1_sb[:], Relu)
                    # first super: small starter DMAs to fill the pipe fast
                    step = PSUM_TILE if (s == 0 and dloc == 0) else dcols
                    for d0 in range(0, dcols, step):
                        di = nc.sync.dma_start(
                            xt[:, d0:d0 + step],
                            xT_d[:, s0 + dloc + d0:s0 + dloc + d0 + step])
                        dep_on_prev(di)
                    if s == 0 and dloc == 0:
                        nc.sync.dma_start(ident_sb[:], ident_d[:])
                    for hloc in range(0, dcols, PSUM_TILE):
                        ps = hp.tile([128, PSUM_TILE], f32, name=f"ps{sx}")
                        for q in range(PSUM_TILE // MM_N):
                            c0 = hloc + q * MM_N
                            nc.tensor.matmul(
                                ps[:, q * MM_N:(q + 1) * MM_N],
                                lhsT=w1_sb[:],
                                rhs=xt[:, c0:c0 + MM_N],
                                start=True, stop=True,
                            )
                        dst = rt[:, dloc + hloc:dloc + hloc + PSUM_TILE]
                        h = (dloc + hloc) // PSUM_TILE
                        if h in dve:
                            nc.vector.tensor_scalar(
                                dst, ps[:], b1_sb[:, 0:1], 0.0,
                                op0=Add, op1=Max)
                        else:
                            nc.scalar.activation(
                                dst, ps[:], Relu, bias=b1_sb[:, 0:1])
                    if s == 0 and dloc + DMA_TILE >= scols:
                        # SWDGE queue (off the HWDGE FIFO), explicitly
                        # held behind super 0's xT stream so the 2 MB
                        # load stays out of the pipeline-fill window;
                        # first consumer (batch 0) runs ~2 supers later
                        from concourse.tile_rust import add_dep_helper
                        ai = nc.gpsimd.dma_start(amap_sb[:], A_d[:])
                        add_dep_helper(ai.ins, di.ins, sync=True,
                                       reason="amap after super-0 stream")
                    if s == n_super - 1:
                        # tail: drain the lagged tree early, then eager
                        # per-DMA-tile trees so little work trails the
                        # final relu
                        while pend_tree:
                            emit_tree(*pend_tree.pop(0))
                            emit_ready_batches(s0 // BLOCK)
                        emit_tree(rt, dloc, dloc + dcols,
                                  (s0 + dloc) // BLOCK)
                if s < n_super - 1:
                    # emit the LAGGED tree (for super s-1) after this
                    # super's relu work is queued, so DVE relus never
                    # wait behind a tree burst
                    while pend_tree:
                        emit_tree(*pend_tree.pop(0))
                        emit_ready_batches(s0 // BLOCK)
                    pend_tree.append((rt, 0, scols, s0 // BLOCK))
            emit_ready_batches(J, force=True)

            # every segment is wholly owned by one core (contiguous runs),
            # so the per-core partial ST is already the FINAL sum for this
            # core's segment window: no collective needed.
            sfull_sb = bp.tile([128, W], f32r, name=f"sfull_sb{sx}")
            nc.scalar.copy(sfull_sb[:], st_ps[:])

          # final: sums_hT = W2.T @ S + b2 x counts; meanT; out
          w2_sb = bp.tile([128, 128], f32r, name=f"w2_sb{sx}")
          b2_sb = bp.tile([1, 128], f32r, name=f"b2_sb{sx}")
          w3_sb = bp.tile([128, 128], f32, name=f"w3_sb{sx}")
          b3_sb = bp.tile([1, 128], f32, name=f"b3_sb{sx}")
          cnt_sb = bp.tile([1, W], f32r, name=f"cnt_sb{sx}")
          inv_sb = bp.tile([128, W], f32, name=f"inv_sb{sx}")
          ones_sb = bp.tile([1, 128], f32, name=f"ones_sb{sx}")
          mean_sb = bp.tile([128, W], f32, name=f"mean_sb{sx}")
          nc.sync.dma_start(w2_sb[:], W2_d[:])
          nc.sync.dma_start(b2_sb[:], b2_d[:])
          nc.sync.dma_start(w3_sb[:], W3_d[:])
          nc.sync.dma_start(b3_sb[:], b3_d[:])
          nc.sync.dma_start(cnt_sb[:], cnt_d[:])
          nc.sync.dma_start(inv_sb[:], inv_d[:])
          nc.sync.dma_start(ones_sb[:], ones_d[:])

          with (
              tc.tile_pool(name=f"sp{sx}", bufs=1, space="PSUM") as sp,
              tc.tile_pool(name=f"op{sx}", bufs=1, space="PSUM") as op,
              tc.tile_pool(name=f"op_sb{sx}", bufs=1) as op_sb,
          ):
            sums_ps = sp.tile([128, W], f32, name=f"sums_ps{sx}")
            nc.tensor.matmul(sums_ps[:, 0:W], lhsT=w2_sb[:],
                             rhs=sfull_sb[:, 0:W], start=True, stop=False)
            nc.tensor.matmul(sums_ps[:, 0:W], lhsT=b2_sb[0:1, :],
                             rhs=cnt_sb[0:1, 0:W], start=False, stop=True)
            nc.vector.tensor_mul(mean_sb[:], sums_ps[:], inv_sb[:])

            n_oc = W // 128
            ops = op.tile([128, n_oc * 128], f32, name=f"ops{sx}")
            for c in range(n_oc):
                c0 = c * 128
                nc.tensor.matmul(ops[:, c0:c0 + 128],
                                 lhsT=mean_sb[:, c0:c0 + 128],
                                 rhs=w3_sb[:], start=True, stop=False)
                nc.tensor.matmul(ops[:, c0:c0 + 128],
                                 lhsT=ones_sb[0:1, :],
                                 rhs=b3_sb[0:1, :], start=False, stop=True)
            osb = op_sb.tile([128, n_oc * 128], f32, name=f"osb{sx}")
            nc.scalar.copy(osb[:], ops[:])
            for c in range(n_oc):
                c0 = c * 128
                last = nc.sync.dma_start(out_d[c0:c0 + 128, :],
                                         osb[:, c0:c0 + 128])
            prev_last[0] = last

        for rep in range(repeat):
            with tc.tile_pool(name=f"body_{rep}", bufs=1) as bp:
                emit_body(rep, bp)

    nc.compile()
    return nc


def _prepare(x1, edge_slices, W1, b1, W2, b2, W3, b3):
    """Host planning + per-core input construction + Bass program build."""
    x1bf = _bf16(np.ascontiguousarray(np.asarray(x1, dtype=np.float32)))
    W1 = np.asarray(W1, dtype=np.float32)
    b1 = np.asarray(b1, dtype=np.float32)
    E = x1bf.shape[0]
    B = int(np.asarray(edge_slices).shape[0]) - 1

    es, counts, seg_blocks, bounds, e_cap = _plan_shards(edge_slices, E, B)
    W1bf = _bf16(W1)
    x_pad = _solve_xpad(W1bf, b1)
    xTs, amaps, J = _build_core_inputs(x1bf, es, counts, seg_blocks, bounds,
                                       e_cap, x_pad, B)

    counts_f = counts.astype(np.float32)
    inv = (1.0 / np.maximum(counts_f, 1.0)).astype(np.float32)
    Wn = WINDOW
    shared = {
        "W1b": W1bf,
        "b1c": np.ascontiguousarray(b1.reshape(128, 1)),
        "W2": np.asarray(W2, dtype=np.float32),
        "b2r": np.ascontiguousarray(np.asarray(b2, np.float32).reshape(1, 128)),
        "W3": np.asarray(W3, dtype=np.float32),
        "b3r": np.ascontiguousarray(np.asarray(b3, np.float32).reshape(1, 128)),
        "ones_row": np.ones((1, 128), np.float32),
        "ident": np.eye(128, dtype=np.float32),
    }

    nc = _build_bass(e_cap, J, B)
    in_maps = []
    for c in range(N_CORES):
        n_c = bounds[c + 1] - bounds[c]
        cnt_w = np.zeros((1, Wn), np.float32)
        cnt_w[0, :n_c] = counts_f[bounds[c]:bounds[c + 1]]
        inv_w = np.zeros((128, Wn), np.float32)
        inv_w[:, :n_c] = inv[None, bounds[c]:bounds[c + 1]]
        in_maps.append({
            "xT": xTs[c], "amap": amaps[c],
            "counts_row": cnt_w, "inv_bcast": np.ascontiguousarray(inv_w),
            **shared,
        })
    return nc, in_maps, bounds


def _assemble(outs, bounds, B):
    out = np.empty((B, 128), dtype=np.float32)
    for c in range(N_CORES):
        n_c = bounds[c + 1] - bounds[c]
        out[bounds[c]:bounds[c + 1], :] = outs[c][:n_c, :]
    return out


def kernel(x1, edge_slices, W1, b1, W2, b2, W3, b3):
    from concourse import bass_utils

    nc, in_maps, bounds = _prepare(x1, edge_slices, W1, b1, W2, b2, W3, b3)
    br = bass_utils.run_bass_kernel_spmd(
        nc, in_maps, core_ids=list(range(N_CORES))
    )
    B = int(np.asarray(edge_slices).shape[0]) - 1
    return _assemble([r["out"] for r in br.results], bounds, B)


# revision 16
# speedup vs baseline: 1.5115x; 1.2439x over previous
"""DeepSet segment-reduce kernel for 8 Trainium2 NeuronCores (Bass/Tile).

Math (reference):
    h  = relu(x1 @ W1 + b1) @ W2 + b2          # [E, 128]
    S  = segment_sum(h, seg)                   # [B, 128]
    mean = S / max(counts, 1)
    out  = mean @ W3 + b3

Because segment-sum is linear, only r = relu(x1 @ W1 + b1) needs per-edge
work:  segsum(h) = segsum(r) @ W2 + counts x b2.

Per-core streaming is HBM-bound at f32 (512 MB of x1), so x1 is cast to
bf16 on the host (rel err ~3e-3 « 2e-2 gate), halving DMA bytes.  The
per-element post-matmul work is then the wall; it is split so no single
engine exceeds the DMA floor:

  host: repack edges so every segment starts on a 32-edge block boundary
        (pad columns use x_pad with W1.T x_pad + b1 < 0, so relu kills
        them exactly), shard contiguous segment runs across 8 cores
        balanced by block count; precompute the one-hot block->local-
        segment matrices A (bf16) so no on-device index math is needed.
  core: stream xT [128, E_cap] bf16 tiles; hT = W1.T @ xT (PE, bf16 W1
        stationary) into [128, 1536] PSUM tiles; bias+relu PSUM->SBUF
        (bf16 out) fused on ACT (~78% of tiles, activation w/ bias) or
        DVE (~22%, tensor_scalar add+max) -- ratio balances the engines;
        per-32-block sums via a pairwise bf16 tensor_tensor tree on DVE
        (2x perf mode; tensor_reduce is capped at 1x), final level f32.
        The tree for super-tile s is emitted after super-tile s+1's DVE
        relu slots so the PSUM pipeline never queues behind a tree burst.
        ST[f, b_local] = block_sumsT @ A via PE transposes + matmuls
        (identical instruction stream on all cores -> one SPMD program);
        every segment is wholly owned by one core (contiguous runs), so
        the per-core partial ST is already final -> no collective; each
        core runs the small W2/W3 stage on its W=256 local window only:
        sums_hT = W2.T @ S + b2 x counts (rank-1 via k=1 matmul);
        meanT   = sums_hT * inv_counts (broadcast tensor input);
        out     = meanT.T @ W3 + 1 x b3 (rank-1), DMA to [W, 128] local
        output; host stitches the 8 disjoint output row ranges.

Self-contained: no reads of /root/problem/*; shapes derived from inputs.
"""

import math

import numpy as np

N_CORES = 8
BLOCK = 32           # segment alignment quantum (edges per block)
PSUM_TILE = 1024     # columns per PSUM tile / relu op (2 banks)
DMA_TILE = 6144      # xT columns per DMA (1.5 MiB bf16)
SUPER = 12288        # columns per block-sum tree pass (12 PSUM tiles)
WINDOW = 256         # per-core local segment window (>= max run + pad)
MM_N = 512           # matmul free-dim chunk (1 PSUM bank of f32)
PAD_MARGIN = 8.0


def _bf16(a):
    import ml_dtypes

    return np.asarray(a).astype(ml_dtypes.bfloat16)


def _plan_shards(edge_slices, E, B):
    es = np.asarray(edge_slices, dtype=np.int64)
    counts = (es[1:] - es[:-1]).astype(np.int64)        # [B]
    seg_blocks = (counts + BLOCK - 1) // BLOCK          # [B]
    total_blocks = int(seg_blocks.sum())

    # contiguous runs of segments per core: exact min-max block partition
    # (binary search the cap + greedy feasibility; every core pays j_max)
    def fit(cap):
        bnds = [0]
        pos = 0
        for c in range(N_CORES):
            acc = 0
            take = 0
            while (pos + take < B and take < WINDOW
                   and acc + int(seg_blocks[pos + take]) <= cap):
                acc += int(seg_blocks[pos + take])
                take += 1
            if take == 0 and pos < B:
                return None
            pos += take
            bnds.append(pos)
        return bnds if pos == B else None

    lo = max(int(seg_blocks.max()),
             (total_blocks + N_CORES - 1) // N_CORES)
    hi = total_blocks
    while lo < hi:
        mid = (lo + hi) // 2
        if fit(mid) is not None:
            hi = mid
        else:
            lo = mid + 1
    bounds = fit(lo)
    assert bounds is not None

    core_blocks = []
    for c in range(N_CORES):
        core_blocks.append(int(seg_blocks[bounds[c]:bounds[c + 1]].sum()))
        assert bounds[c + 1] - bounds[c] <= WINDOW, (
            "segment run exceeds local window", bounds)
    j_max = max(core_blocks)
    e_cap = int(math.ceil(j_max * BLOCK / PSUM_TILE) * PSUM_TILE)
    return es, counts, seg_blocks, bounds, e_cap


def _solve_xpad(W1bf, b1):
    # x_pad (bf16) with W1.T x_pad + b1 <= -1 elementwise => relu output 0
    W1d = W1bf.astype(np.float64)
    margin = PAD_MARGIN
    for _ in range(6):
        rhs = -(b1.astype(np.float64) + margin)
        x_pad = _bf16(np.linalg.solve(W1d.T, rhs))
        chk = W1d.T @ x_pad.astype(np.float64) + b1.astype(np.float64)
        if chk.max() < -1.0:
            return x_pad
        margin *= 2.0
    raise AssertionError("x_pad margin too small")


def _build_core_inputs(x1bf, es, counts, seg_blocks, bounds, e_cap, x_pad, B):
    J = e_cap // BLOCK
    n_chunks = (J + 127) // 128
    xTs, amaps = [], []
    for c in range(N_CORES):
        xT = np.empty((128, e_cap), dtype=x1bf.dtype)
        segid = np.full(n_chunks * 128, -1, dtype=np.int64)
        pos = 0
        for b in range(bounds[c], bounds[c + 1]):
            cnt = int(counts[b])
            if cnt == 0:
                continue
            xT[:, pos:pos + cnt] = x1bf[es[b]:es[b + 1], :].T
            nb = int(seg_blocks[b])
            pad = nb * BLOCK - cnt
            if pad:
                xT[:, pos + cnt:pos + nb * BLOCK] = x_pad[:, None]
            segid[pos // BLOCK: pos // BLOCK + nb] = b - bounds[c]
            pos += nb * BLOCK
        if pos < e_cap:
            xT[:, pos:] = x_pad[:, None]
        # one-hot A: block (chunk k, row p) -> local segment column
        amap = np.zeros((128, n_chunks * WINDOW), np.float32)
        for k in range(n_chunks):
            sid = segid[k * 128:(k + 1) * 128]
            p = np.nonzero(sid >= 0)[0]
            amap[p, k * WINDOW + sid[p]] = 1.0
        xTs.append(xT)
        amaps.append(_bf16(amap))
    return xTs, amaps, J


def _dve_slots(s):
    # DVE-relu sub-tile positions within super-tile s (rest go to ACT);
    # spread out; avg 2.5/12 balances ACT(relu) vs DVE(relu+tree).
    return (3, 7, 11) if s % 2 else (7, 11)


def _build_bass(e_cap, J, B, repeat=1):
    n_chunks = (J + 127) // 128
    n_batches = (n_chunks + 3) // 4
    W = WINDOW
    import concourse.bacc as bacc
    import concourse.mybir as mybir
    import concourse.tile as tile

    f32 = mybir.dt.float32
    f32r = mybir.dt.float32r
    bf16 = mybir.dt.bfloat16
    Relu = mybir.ActivationFunctionType.Relu
    Add = mybir.AluOpType.add
    Max = mybir.AluOpType.max

    nc = bacc.Bacc(trn_type="TRN2", num_devices=N_CORES)

    xT_d = nc.dram_tensor("xT", [128, e_cap], bf16, kind="ExternalInput")
    A_d = nc.dram_tensor("amap", [128, n_chunks * W], bf16,
                         kind="ExternalInput")
    W1_d = nc.dram_tensor("W1b", [128, 128], bf16, kind="ExternalInput")
    b1_d = nc.dram_tensor("b1c", [128, 1], f32, kind="ExternalInput")
    W2_d = nc.dram_tensor("W2", [128, 128], f32r, kind="ExternalInput")
    b2_d = nc.dram_tensor("b2r", [1, 128], f32r, kind="ExternalInput")
    W3_d = nc.dram_tensor("W3", [128, 128], f32, kind="ExternalInput")
    b3_d = nc.dram_tensor("b3r", [1, 128], f32, kind="ExternalInput")
    cnt_d = nc.dram_tensor("counts_row", [1, W], f32r, kind="ExternalInput")
    inv_d = nc.dram_tensor("inv_bcast", [128, W], f32, kind="ExternalInput")
    ones_d = nc.dram_tensor("ones_row", [1, 128], f32, kind="ExternalInput")
    ident_d = nc.dram_tensor("ident", [128, 128], f32, kind="ExternalInput")
    out_d = nc.dram_tensor("out", [W, 128], f32, kind="ExternalOutput")

    n_super = (e_cap + SUPER - 1) // SUPER

    with tile.TileContext(nc) as tc, tc.tile_pool(name="persist", bufs=1):
        prev_last = [None]

        def emit_body(rep, bp):
          # one full pass of the kernel; rep > 0 only exists for the
          # repeat-timing harness (same work re-emitted, serialized on rep-1)
          sx = f"_{rep}" if rep else ""

          def dep_on_prev(inst):
              if prev_last[0] is not None:
                  from concourse.tile_rust import add_dep_helper
                  add_dep_helper(inst.ins, prev_last[0].ins, sync=True,
                                 reason="repeat-timing serialization")

          w1_sb = bp.tile([128, 128], bf16, name=f"w1_sb{sx}")
          b1_sb = bp.tile([128, 1], f32, name=f"b1_sb{sx}")
          ident_sb = bp.tile([128, 128], f32, name=f"ident_sb{sx}")
          amap_sb = bp.tile([128, n_chunks * W], bf16, name=f"amap_sb{sx}")
          warm_sb = bp.tile([128, 1], f32, name=f"warm_sb{sx}")

          with (
              tc.tile_pool(name=f"xp{sx}", bufs=4) as xp,
              tc.tile_pool(name=f"rp{sx}", bufs=3) as rp,
              tc.tile_pool(name=f"sc{sx}", bufs=1) as scp,
              tc.tile_pool(name=f"hp{sx}", bufs=3, space="PSUM") as hp,
              tc.tile_pool(name=f"tp{sx}", bufs=1, space="PSUM") as tp,
              tc.tile_pool(name=f"stp{sx}", bufs=1, space="PSUM") as stp,
          ):
            st_ps = stp.tile([128, W], f32, name=f"st_ps{sx}")
            scrA = scp.tile([128, SUPER // 2], bf16, name=f"scrA{sx}")
            scrB = scp.tile([128, SUPER // 4], bf16, name=f"scrB{sx}")
            bs_sb = scp.tile([128, n_chunks * 128], f32, name=f"bs_sb{sx}")
            bsT_sb = scp.tile([128, n_batches * 512], bf16,
                              name=f"bsT_sb{sx}")

            def emit_batch(b):
                # 4 PE transposes of bs chunks -> one PSUM tile -> one copy
                # to bsT (bf16), then the block->segment matmuls.
                k0, k1 = b * 4, min(b * 4 + 4, n_chunks)
                tpb = tp.tile([128, 512], f32, name=f"tpb{sx}")
                for k in range(k0, k1):
                    jw = min(128, J - k * 128)
                    s = (k - k0) * 128
                    nc.tensor.transpose(
                        tpb[:jw, s:s + 128],
                        bs_sb[:, k * 128:k * 128 + jw], ident_sb[:])
                if b % 2:
                    nc.scalar.copy(bsT_sb[:, b * 512:(b + 1) * 512], tpb[:])
                else:
                    nc.vector.tensor_copy(
                        bsT_sb[:, b * 512:(b + 1) * 512], tpb[:])
                for k in range(k0, k1):
                    jw = min(128, J - k * 128)
                    nc.tensor.matmul(
                        st_ps[:, 0:W],
                        lhsT=bsT_sb[:jw, k * 128:(k + 1) * 128],
                        rhs=amap_sb[:jw, k * W:(k + 1) * W],
                        start=(k == 0), stop=(k == n_chunks - 1),
                    )

            def emit_tree(rt, c0, c1, j0):
                # per-32-block sums over rt[:, c0:c1]: pairwise adds, bf16
                # intermediates at 2x DVE rate, final level f32 into bs_sb
                j = (c1 - c0) // BLOCK

                def lvl(src, dst, w, n):
                    si = src.rearrange("p (j w) -> p j w", w=w)[:, 0:n, :]
                    do = dst.rearrange(
                        "p (j w) -> p j w", w=w // 2)[:, 0:n, :]
                    nc.vector.tensor_tensor(
                        do, si[:, :, 0:w // 2], si[:, :, w // 2:w], op=Add)

                lvl(rt[:, c0:c1], scrA[:, 0:j * 16], 32, j)
                lvl(scrA[:, 0:j * 16], scrB[:, 0:j * 8], 16, j)
                lvl(scrB[:, 0:j * 8], scrA[:, 0:j * 4], 8, j)
                lvl(scrA[:, 0:j * 4], scrB[:, 0:j * 2], 4, j)
                s2 = scrB[:, 0:j * 2].rearrange("p (j w) -> p j w", w=2)
                bsw = bs_sb[:, j0:j0 + j].rearrange("p (j w) -> p j w", w=1)
                nc.vector.tensor_tensor(
                    bsw, s2[:, :, 0:1], s2[:, :, 1:2], op=Add)

            next_batch = [0]

            def emit_ready_batches(blocks_done, force=False):
                while next_batch[0] < n_batches:
                    b = next_batch[0]
                    hi = min((b * 4 + 4) * 128, J)
                    if not force and blocks_done < hi:
                        break
                    emit_batch(b)
                    next_batch[0] += 1

            pend_tree = []   # (rt, c0, c1, j0) lagged one super-tile

            for s in range(n_super):
                s0 = s * SUPER
                scols = min(SUPER, e_cap - s0)
                rt = rp.tile([128, SUPER], bf16, name=f"rt{sx}")
                dve = _dve_slots(s)
                for dloc in range(0, scols, DMA_TILE):
                    xt = xp.tile([128, DMA_TILE], bf16, name=f"xt{sx}")
                    dcols = min(DMA_TILE, scols - dloc)
                    if s == 0 and dloc == 0:
                        # load weights first + trigger ACT table load early
                        wd = nc.sync.dma_start(w1_sb[:], W1_d[:])
                        dep_on_prev(wd)
                        wd = nc.sync.dma_start(b1_sb[:], b1_d[:])
                        dep_on_prev(wd)
                        nc.scalar.activation(warm_sb[:], b1_sb[:], Relu)
                    # first super: small starter DMAs to fill the pipe fast
                    step = PSUM_TILE if (s == 0 and dloc == 0) else dcols
                    for d0 in range(0, dcols, step):
                        di = nc.sync.dma_start(
                            xt[:, d0:d0 + step],
                            xT_d[:, s0 + dloc + d0:s0 + dloc + d0 + step])
                        dep_on_prev(di)
                    if s == 0 and dloc == 0:
                        nc.sync.dma_start(ident_sb[:], ident_d[:])
                    for hloc in range(0, dcols, PSUM_TILE):
                        ps = hp.tile([128, PSUM_TILE], f32, name=f"ps{sx}")
                        for q in range(PSUM_TILE // MM_N):
                            c0 = hloc + q * MM_N
                            nc.tensor.matmul(
                                ps[:, q * MM_N:(q + 1) * MM_N],
                                lhsT=w1_sb[:],
                                rhs=xt[:, c0:c0 + MM_N],
                                start=True, stop=True,
                            )
                        dst = rt[:, dloc + hloc:dloc + hloc + PSUM_TILE]
                        h = (dloc + hloc) // PSUM_TILE
                        if h in dve:
                            nc.vector.tensor_scalar(
                                dst, ps[:], b1_sb[:, 0:1], 0.0,
                                op0=Add, op1=Max)
                        else:
                            nc.scalar.activation(
                                dst, ps[:], Relu, bias=b1_sb[:, 0:1])
                    if s == 0 and dloc + DMA_TILE >= scols:
                        # SWDGE queue (off the HWDGE FIFO), explicitly
                        # held behind super 0's xT stream so the 2 MB
                        # load stays out of the pipeline-fill window;
                        # first consumer (batch 0) runs ~2 supers later
                        from concourse.tile_rust import add_dep_helper
                        ai = nc.gpsimd.dma_start(amap_sb[:], A_d[:])
                        add_dep_helper(ai.ins, di.ins, sync=True,
                                       reason="amap after super-0 stream")
                    if s == n_super - 1:
                        # tail: drain the lagged tree early, then eager
                        # per-DMA-tile trees so little work trails the
                        # final relu
                        while pend_tree:
                            emit_tree(*pend_tree.pop(0))
                            emit_ready_batches(s0 // BLOCK)
                        emit_tree(rt, dloc, dloc + dcols,
                                  (s0 + dloc) // BLOCK)
                if s < n_super - 1:
                    # emit the LAGGED tree (for super s-1) after this
                    # super's relu work is queued, so DVE relus never
                    # wait behind a tree burst
                    while pend_tree:
                        emit_tree(*pend_tree.pop(0))
                        emit_ready_batches(s0 // BLOCK)
                    pend_tree.append((rt, 0, scols, s0 // BLOCK))
            emit_ready_batches(J, force=True)

            # every segment is wholly owned by one core (contiguous runs),
            # so the per-core partial ST is already the FINAL sum for this
            # core's segment window: no collective needed.
            sfull_sb = bp.tile([128, W], f32r, name=f"sfull_sb{sx}")
            nc.scalar.copy(sfull_sb[:], st_ps[:])

          # final: sums_hT = W2.T @ S + b2 x counts; meanT; out
          w2_sb = bp.tile([128, 128], f32r, name=f"w2_sb{sx}")
          b2_sb = bp.tile([1, 128], f32r, name=f"b2_sb{sx}")
          w3_sb = bp.tile([128, 128], f32, name=f"w3_sb{sx}")
          b3_sb = bp.tile([1, 128], f32, name=f"b3_sb{sx}")
          cnt_sb = bp.tile([1, W], f32r, name=f"cnt_sb{sx}")
          inv_sb = bp.tile([128, W], f32, name=f"inv_sb{sx}")
          ones_sb = bp.tile([1, 128], f32, name=f"ones_sb{sx}")
          mean_sb = bp.tile([128, W], f32, name=f"mean_sb{sx}")
          nc.sync.dma_start(w2_sb[:], W2_d[:])
          nc.sync.dma_start(b2_sb[:], b2_d[:])
          nc.sync.dma_start(w3_sb[:], W3_d[:])
          nc.sync.dma_start(b3_sb[:], b3_d[:])
          nc.sync.dma_start(cnt_sb[:], cnt_d[:])
          nc.sync.dma_start(inv_sb[:], inv_d[:])
          nc.sync.dma_start(ones_sb[:], ones_d[:])

          with (
              tc.tile_pool(name=f"sp{sx}", bufs=1, space="PSUM") as sp,
              tc.tile_pool(name=f"op{sx}", bufs=1, space="PSUM") as op,
              tc.tile_pool(name=f"op_sb{sx}", bufs=1) as op_sb,
          ):
            sums_ps = sp.tile([128, W], f32, name=f"sums_ps{sx}")
            nc.tensor.matmul(sums_ps[:, 0:W], lhsT=w2_sb[:],
                             rhs=sfull_sb[:, 0:W], start=True, stop=False)
            nc.tensor.matmul(sums_ps[:, 0:W], lhsT=b2_sb[0:1, :],
                             rhs=cnt_sb[0:1, 0:W], start=False, stop=True)
            nc.vector.tensor_mul(mean_sb[:], sums_ps[:], inv_sb[:])

            n_oc = W // 128
            ops = op.tile([128, n_oc * 128], f32, name=f"ops{sx}")
            for c in range(n_oc):
                c0 = c * 128
                nc.tensor.matmul(ops[:, c0:c0 + 128],
                                 lhsT=mean_sb[:, c0:c0 + 128],
                                 rhs=w3_sb[:], start=True, stop=False)
                nc.tensor.matmul(ops[:, c0:c0 + 128],
                                 lhsT=ones_sb[0:1, :],
                                 rhs=b3_sb[0:1, :], start=False, stop=True)
            osb = op_sb.tile([128, n_oc * 128], f32, name=f"osb{sx}")
            nc.scalar.copy(osb[:], ops[:])
            for c in range(n_oc):
                c0 = c * 128
                last = nc.sync.dma_start(out_d[c0:c0 + 128, :],
                                         osb[:, c0:c0 + 128])
            prev_last[0] = last

        for rep in range(repeat):
            with tc.tile_pool(name=f"body_{rep}", bufs=1) as bp:
                emit_body(rep, bp)

    nc.compile()
    return nc


def _prepare(x1, edge_slices, W1, b1, W2, b2, W3, b3):
    """Host planning + per-core input construction + Bass program build."""
    x1bf = _bf16(np.ascontiguousarray(np.asarray(x1, dtype=np.float32)))
    W1 = np.asarray(W1, dtype=np.float32)
    b1 = np.asarray(b1, dtype=np.float32)
    E = x1bf.shape[0]
    B = int(np.asarray(edge_slices).shape[0]) - 1

    es, counts, seg_blocks, bounds, e_cap = _plan_shards(edge_slices, E, B)
    W1bf = _bf16(W1)
    x_pad = _solve_xpad(W1bf, b1)
    xTs, amaps, J = _build_core_inputs(x1bf, es, counts, seg_blocks, bounds,
                                       e_cap, x_pad, B)

    counts_f = counts.astype(np.float32)
    inv = (1.0 / np.maximum(counts_f, 1.0)).astype(np.float32)
    Wn = WINDOW
    shared = {
        "W1b": W1bf,
        "b1c": np.ascontiguousarray(b1.reshape(128, 1)),
        "W2": np.asarray(W2, dtype=np.float32),
        "b2r": np.ascontiguousarray(np.asarray(b2, np.float32).reshape(1, 128)),
        "W3": np.asarray(W3, dtype=np.float32),
        "b3r": np.ascontiguousarray(np.asarray(b3, np.float32).reshape(1, 128)),
        "ones_row": np.ones((1, 128), np.float32),
        "ident": np.eye(128, dtype=np.float32),
    }

    nc = _build_bass(e_cap, J, B)
    in_maps = []
    for c in range(N_CORES):
        n_c = bounds[c + 1] - bounds[c]
        cnt_w = np.zeros((1, Wn), np.float32)
        cnt_w[0, :n_c] = counts_f[bounds[c]:bounds[c + 1]]
        inv_w = np.zeros((128, Wn), np.float32)
        inv_w[:, :n_c] = inv[None, bounds[c]:bounds[c + 1]]
        in_maps.append({
            "xT": xTs[c], "amap": amaps[c],
            "counts_row": cnt_w, "inv_bcast": np.ascontiguousarray(inv_w),
            **shared,
        })
    return nc, in_maps, bounds


def _assemble(outs, bounds, B):
    out = np.empty((B, 128), dtype=np.float32)
    for c in range(N_CORES):
        n_c = bounds[c + 1] - bounds[c]
        out[bounds[c]:bounds[c + 1], :] = outs[c][:n_c, :]
    return out


def kernel(x1, edge_slices, W1, b1, W2, b2, W3, b3):
    from concourse import bass_utils

    nc, in_maps, bounds = _prepare(x1, edge_slices, W1, b1, W2, b2, W3, b3)
    br = bass_utils.run_bass_kernel_spmd(
        nc, in_maps, core_ids=list(range(N_CORES))
    )
    B = int(np.asarray(edge_slices).shape[0]) - 1
    return _assemble([r["out"] for r in br.results], bounds, B)
